# revision 94
# baseline (speedup 1.0000x reference)
"""AdaptiveSpanAttention Trainium2 kernel (8 NeuronCores).

Sharding: core c -> (batch b = c//2, head-group g = c%2).
Each core computes, for its batch and its 8 heads:
  Q/K/V projections, anti-causal (j>=i) attention with adaptive-span
  mask, renormalization, and a partial output projection
  y_part = Out_g @ Wo[:, e_slice].T  (contraction over its 512 channels).
Host combines: y[b] = y_part[2b] + y_part[2b+1] + bo.

Q/K/V projections run as fp8e4 DoubleRow matmuls with a hi/lo split
(x = xh + xl, W = wh + wl after power-of-2 pre-scaling; the three
products xh*wh, xl*wh, xh*wl share one scale so they accumulate into a
single PSUM group at 0.75x the bf16 instruction cost). The descale
factors fold into the exp scale (scores) and the finalize multiplier
(V path), so no extra vector work is added. Attention itself stays
bf16 (64-deep score contraction cannot pack DoubleRow slots).

The span network z = T*sigmoid(mean_t x @ Wspan + bspan) is computed
on host (it already ran there for mask-width specialization); the
kernel receives pre-clamped span-ramp tiles mt[h,k] and multiplies
them in directly.

Schedule notes:
- host packs x / weights into hi/lo fp8 mega-tiles ([128, DT, T] /
  [128, DT, E]) so each tensor is 1-2 large contiguous DMAs, ordered so
  V -> Q -> K groups start as their operands land.
- attention is software-pipelined: scores for block st+1 are issued
  before attn@V of block st so the exp+mask chain hides under PE work;
  one mask multiply covers both heads of a pair (causal on Pool, wide
  span ramps on DVE).
- span bounds specialized per call from host-computed z: mask ops only
  on the ramp band, fully-masked score columns skipped.
"""
import sys

sys.path.insert(0, "/opt/trn_rl_repo")

from contextlib import ExitStack

import ml_dtypes
import numpy as np

import concourse.bass as bass
import concourse.tile as tile
from concourse import bacc, mybir
from concourse.bass_utils import run_bass_kernel_spmd

BF16 = mybir.dt.bfloat16
F16 = mybir.dt.float16
F32 = mybir.dt.float32
FP8 = mybir.dt.float8e4
DR = mybir.MatmulPerfMode.DoubleRow
E4NP = ml_dtypes.float8_e4m3fn

B, T, D, H = 4, 1024, 1024, 16
DH = 64          # head dim
R = 256.0
HC = 8           # heads per core
E = 512          # channels per core (HC * DH)
N_CORES = 8
TCH = 512        # t-chunk width (PSUM f32 free-dim limit)
NT = T // TCH    # 2 t-chunks
ST = T // 128    # 8 s-tiles
DT = D // 128    # 8 d-tiles

SX = 32.0        # x pre-scale (|x| < 6 -> |x*SX| < 200, e4m3 max 448)
SW = 1024.0      # W pre-scale (|W| < 0.2)
DESCALE = 1.0 / (SX * SW)

_NC_CACHE = {}


def causal_width(st, tch):
    """Valid query-column width of block (s_tile=st, t_chunk=tch)."""
    delta = 128 * st - 512 * tch
    return max(0, min(TCH, delta + 128))


def span_width(st, tch, zlo):
    """Columns [0, m_w) where the span mask can differ from 1 (z >= zlo)."""
    delta = 128 * st - 512 * tch
    w = causal_width(st, tch)
    return max(0, min(w, delta + 127 - int(zlo)))


def dead_width(st, tch, zhi):
    """Columns [0, d_w) where the span mask is identically 0 (z <= zhi)."""
    delta = 128 * st - 512 * tch
    w = causal_width(st, tch)
    return max(0, min(w, int(delta - R - zhi) // 16 * 16))


def _mt_layout(zlo, zhi, skip384):
    """(offsets dict {(k): (off, d_w, m_w)}, total cols) of the packed
    per-head span-ramp table; per-head stride is the total."""
    offs = {}
    off = 0
    for k in range(ST):
        m_w = span_width(k, 0, zlo)
        if m_w <= 0 or (k == 3 and skip384):
            continue
        d_w = dead_width(k, 0, zhi)
        if m_w <= d_w:
            continue
        offs[k] = (off, d_w, m_w)
        off += m_w - d_w
    return offs, off


def build_nc(zlo, zhi, skip384):
    key = (zlo, zhi, skip384)
    if key in _NC_CACHE:
        return _NC_CACHE[key]
    nc = bacc.Bacc("TRN2", target_bir_lowering=False, debug=False, num_devices=1)

    mt_offs, mt_cols = _mt_layout(zlo, zhi, skip384)

    # ---- DRAM parameters (per-core shards prepared on host) ----
    # hi/lo fp8 split tensors in [128, DT, free] mega-tile layout; DoubleRow
    # slots pair adjacent d-tiles of one product type (classic [P, 2, M]
    # stationary APs that walrus accepts; all slices DMA-contiguous)
    xh_d = nc.declare_dram_parameter("xh8", [128, DT, T], FP8, isOutput=False)
    xl_d = nc.declare_dram_parameter("xl8", [128, DT, T], FP8, isOutput=False)
    Wqh_d = nc.declare_dram_parameter("Wqh8", [128, DT, E], FP8, isOutput=False)
    Wql_d = nc.declare_dram_parameter("Wql8", [128, DT, E], FP8, isOutput=False)
    Wkh_d = nc.declare_dram_parameter("Wkh8", [128, DT, E], FP8, isOutput=False)
    Wkl_d = nc.declare_dram_parameter("Wkl8", [128, DT, E], FP8, isOutput=False)
    Wvh_d = nc.declare_dram_parameter("Wvh8", [128, DT, E], FP8, isOutput=False)
    Wvl_d = nc.declare_dram_parameter("Wvl8", [128, DT, E], FP8, isOutput=False)
    Wo_d = nc.declare_dram_parameter("Wo8", [128, D, 4], BF16, isOutput=False)
    # packed span-ramp tiles: mt8[p, h, off_k + j] = clamp((R+z_h-d)/R, 0, 1)
    # for d = 128k + p - (d_w + j); 0 where causal-invalid
    mt_d = nc.declare_dram_parameter("mt8", [128, HC, max(1, mt_cols)],
                                     F16, isOutput=False)
    # c01[s', k, j] = 1.0 if s' >= j else 0.0  (causal 0/1 for t' = 128k + j)
    c01_d = nc.declare_dram_parameter("c01", [128, 4, 128], F16, isOutput=False)
    yp_d = nc.declare_dram_parameter("yp", [T, D], F16, isOutput=True)

    with tile.TileContext(nc) as tc, ExitStack() as ctx:
        # ---------------- pools ----------------
        consts = ctx.enter_context(tc.tile_pool(name="consts", bufs=1))
        xp = ctx.enter_context(tc.tile_pool(name="xp", bufs=1))
        wp = ctx.enter_context(tc.tile_pool(name="wp", bufs=1))
        qkp = ctx.enter_context(tc.tile_pool(name="qkp", bufs=1))
        vp = ctx.enter_context(tc.tile_pool(name="vp", bufs=1))
        outp = ctx.enter_context(tc.tile_pool(name="outp", bufs=1))
        scr = ctx.enter_context(tc.tile_pool(name="scr", bufs=3))
        ysb = ctx.enter_context(tc.tile_pool(name="ysb", bufs=6))

        lead_ctx = ExitStack()
        ps_lead = lead_ctx.enter_context(
            tc.tile_pool(name="ps_lead", bufs=8, space="PSUM"))

        # ---------------- loads (few large DMAs) ----------------
        xh = xp.tile([128, DT, T], FP8, name="xh8")
        xl = xp.tile([128, DT, T], FP8, name="xl8")
        wqh = wp.tile([128, DT, E], FP8, name="wqh8")
        wql = wp.tile([128, DT, E], FP8, name="wql8")
        wkh = wp.tile([128, DT, E], FP8, name="wkh8")
        wkl = wp.tile([128, DT, E], FP8, name="wkl8")
        wvh = wp.tile([128, DT, E], FP8, name="wvh8")
        wvl = wp.tile([128, DT, E], FP8, name="wvl8")
        wo = wp.tile([128, D, 4], BF16, name="wo8")
        # DMA order tuned to the lead emission: V groups first (largest PE
        # block), then Q t-chunks, then K; mt/c01/wo only feed the
        # attention phase and stream last
        # first two tensors stream in dt-halves so the first V matmuls
        # start ~1.5us sooner
        nc.sync.dma_start(xh[:, 0:4, 0:TCH], xh_d[:, 0:4, 0:TCH])
        nc.sync.dma_start(wvh[:, 0:4, :], Wvh_d[:, 0:4, :])
        nc.sync.dma_start(xh[:, 4:DT, 0:TCH], xh_d[:, 4:DT, 0:TCH])
        nc.sync.dma_start(wvh[:, 4:DT, :], Wvh_d[:, 4:DT, :])
        nc.sync.dma_start(xl[:, :, 0:TCH], xl_d[:, :, 0:TCH])
        nc.sync.dma_start(wvl[:], Wvl_d[:, :, :])
        nc.sync.dma_start(wqh[:], Wqh_d[:, :, :])
        nc.sync.dma_start(wql[:], Wql_d[:, :, :])
        nc.sync.dma_start(xh[:, :, TCH:T], xh_d[:, :, TCH:T])
        nc.sync.dma_start(wkh[:], Wkh_d[:, :, :])
        nc.sync.dma_start(xl[:, :, TCH:T], xl_d[:, :, TCH:T])
        nc.sync.dma_start(wkl[:], Wkl_d[:, :, :])
        c01_sb = consts.tile([128, 4, 128], F16, tag="c01")
        nc.sync.dma_start(c01_sb[:], c01_d[:, :, :])
        # mt sliced per head-pair: with attention starting ~21.5us, pair
        # (0,0)'s first ramp block would otherwise race the full-table DMA
        mt_sb = consts.tile([128, HC, max(1, mt_cols)], F16, tag="mt8")
        nc.sync.dma_start(mt_sb[:, 0:2, :], mt_d[:, 0:2, :])
        nc.sync.dma_start(mt_sb[:, 2:4, :], mt_d[:, 2:4, :])
        nc.sync.dma_start(wo[:], Wo_d[:, :, :])
        nc.sync.dma_start(mt_sb[:, 4:6, :], mt_d[:, 4:6, :])
        nc.sync.dma_start(mt_sb[:, 6:8, :], mt_d[:, 6:8, :])

        # ---------------- Q/K projections (transposed layout) ----------------
        # QT[e, t] = sum_d W[d, e] * xT[d, t] in fp8 DoubleRow 3-term:
        # per dtile (wh)x(xh,xl), per dtile-pair (wl_d,wl_d+1)x(xh_d,xh_d+1)
        qt_sb = [qkp.tile([128, T], BF16, tag="qt", name=f"qt{i}", bufs=4)
                 for i in range(4)]
        kt_sb = [qkp.tile([128, T], BF16, tag="kt", name=f"kt{i}", bufs=4)
                 for i in range(4)]

        def proj_mms(ps, w_hl, et, t0, t1):
            w_hi, w_lo = w_hl
            eb = 128 * et
            w_cols = t1 - t0
            for i, (wt, xt) in enumerate(
                    ((w_hi, xh), (w_hi, xl), (w_lo, xh))):
                for dt_i in range(0, DT, 2):
                    nc.tensor.matmul(
                        ps[:, 0:w_cols],
                        wt[:, dt_i:dt_i + 2, eb:eb + 128],
                        xt[:, dt_i:dt_i + 2, t0:t1],
                        start=(i == 0 and dt_i == 0),
                        stop=(i == 2 and dt_i == DT - 2), perf_mode=DR)

        def emit_proj(dst, w8, et, t0, t1, copy_eng="act", pool=None):
            pool = pool or ps_lead
            ps = pool.tile([128, TCH], F32, tag="pj", name=f"pj{et}_{t0}",
                           padded_shape=[128, TCH])
            proj_mms(ps, w8, et, t0, t1)
            if copy_eng == "act":
                nc.scalar.copy(dst[et][:, t0:t1], ps[:, 0:t1 - t0])
            else:
                nc.vector.tensor_copy(dst[et][:, t0:t1], ps[:, 0:t1 - t0])

        WQ = (wqh, wql)
        WK = (wkh, wkl)

        # ---------------- V (natural layout, ones-augmented) ----------------
        # v_aug[st][p, h, 0:64] = V_raw[128*st+p, 64h+j] (scaled SX*SW);
        # v_aug[st][p, h, 64:128] = 1 (exact denominator rows; the V descale
        # folds into the finalize multiplier)
        v_aug = [None] * ST

        def emit_v(st, pool=None, copy_eng="dve"):
            pool = pool or ps_lead
            va = vp.tile([128, HC, 2 * DH], BF16, tag="vaug", bufs=ST,
                         name=f"vaug{st}")
            nc.gpsimd.memset(va[:, :, DH:2 * DH], 1.0)
            ps = pool.tile([128, E], F32, tag="pj", name=f"pjv{st}")
            sb = 128 * st
            for i, (xt, wt) in enumerate(
                    ((xh, wvh), (xl, wvh), (xh, wvl))):
                for dt_i in range(0, DT, 2):
                    nc.tensor.matmul(
                        ps[:],
                        xt[:, dt_i:dt_i + 2, sb:sb + 128],
                        wt[:, dt_i:dt_i + 2, :],
                        start=(i == 0 and dt_i == 0),
                        stop=(i == 2 and dt_i == DT - 2), perf_mode=DR)
            ceng = nc.scalar.copy if copy_eng == "act" else nc.vector.tensor_copy
            ceng(va[:, :, 0:DH], ps[:].rearrange("p (h d) -> p h d", h=HC))
            v_aug[st] = va

        # lead emission ordered by DMA arrival: V st0-5 first (x + wv),
        # then Q chunks, then K et0 only -- K et1, V6, V7 move into pair
        # (0,0)'s fillers so attention starts ~5us earlier and that work
        # lands in the attention phase's stall pockets
        for st in range(4):
            emit_v(st, copy_eng=("dve" if st < 2 else "act"))
        emit_proj(qt_sb, WQ, 0, 0, TCH)
        emit_proj(qt_sb, WQ, 1, 0, TCH)
        emit_proj(qt_sb, WQ, 0, TCH, T)
        emit_proj(qt_sb, WQ, 1, TCH, T)
        # K et0 copies go on DVE: they land right at attention start and
        # must not queue ahead of the first exp ops on ACT
        for tch in range(NT):
            emit_proj(kt_sb, WK, 0, TCH * tch, TCH * (tch + 1),
                      copy_eng="dve")

        # spare projection work at the lead tail keeps PE busy while the
        # first score block's psum bank clears its lead-phase WAR; two
        # half-width groups make the last copy (the WAR reader) short
        ps_sp1 = ps_lead.tile([128, 256], F32, tag="pj", name="pjsp1",
                              padded_shape=[128, TCH])
        proj_mms(ps_sp1, WQ, 2, 0, 256)
        nc.scalar.copy(qt_sb[2][:, 0:256], ps_sp1[:, 0:256])
        ps_sp2 = ps_lead.tile([128, 256], F32, tag="pj", name="pjsp2",
                              padded_shape=[128, TCH])
        proj_mms(ps_sp2, WQ, 2, 256, TCH)
        nc.vector.tensor_copy(qt_sb[2][:, 256:TCH], ps_sp2[:, 0:256])

        lead_ctx.close()
        attn_ctx = ExitStack()
        ps_sc = attn_ctx.enter_context(
            tc.tile_pool(name="ps_sc", bufs=2, space="PSUM"))
        ps_out = attn_ctx.enter_context(
            tc.tile_pool(name="ps_out", bufs=2, space="PSUM"))
        ps_fill = attn_ctx.enter_context(
            tc.tile_pool(name="ps_fill", bufs=2, space="PSUM"))

        def proj_fillers(specs):
            """Filler closures whose psum->SBUF copy is deferred one slot so
            it queues behind the current block's exp/mask, not ahead."""
            fs = []
            pend = [None]

            def make(dst, w8, et, tch):
                def f():
                    ps = ps_fill.tile([128, TCH], F32, tag="pj",
                                      name=f"pjf{et}_{tch}",
                                      padded_shape=[128, TCH])
                    proj_mms(ps, w8, et, TCH * tch, TCH * (tch + 1))
                    prev, pend[0] = pend[0], (
                        lambda: nc.vector.tensor_copy(
                            dst[et][:, TCH * tch:TCH * (tch + 1)], ps[:]))
                    if prev is not None:
                        prev()
                return f

            for dst, w8, et, tch in specs:
                fs.append(make(dst, w8, et, tch))

            def flush():
                if pend[0] is not None:
                    pend[0]()
                    pend[0] = None
            fs.append(flush)
            return fs

        # ---------------- attention ----------------
        # out_pair[j][tch] holds heads 2j (parts 0:64) and 2j+1 (parts 64:128)
        out_pair = [[outp.tile([128, TCH], BF16, tag="out", bufs=8,
                               name=f"op{j}_{c}") for c in range(NT)]
                    for j in range(4)]

        # exp absorbs the Q/K descales: p = exp(s_raw / (8 * (SX*SW)^2))
        EXP_SCALE = 1.0 / (8.0 * (SX * SW) ** 2)

        def attn_pair(tch, j, v_prefetch=False, fillers=(), prev_fin=None,
                      split_fin=False, split_exp=False, fin_idx=0):
            """Attention for head pair (2j, 2j+1); both share et=j.

            Scores for the two heads go into one 2-bank psum pair-tile so a
            single exp covers both. Scores run one block ahead of attn@V so
            the exp+mask chain hides under PE work. The previous pair's
            out-division (prev_fin) is emitted after this pair's first score
            block so it does not wedge ahead of this pair's mask ops in the
            DVE queue. Returns this pair's finalize closure.
            """
            first_st = 4 * tch
            heads = (2 * j, 2 * j + 1)
            pouts = [ps_out.tile([128, TCH], F32, tag="pout",
                                 name=f"pout{h}_{tch}") for h in heads]
            fillers = list(fillers)
            p_tiles = {}

            def block_ranges(st):
                w = causal_width(st, tch)
                d_w = dead_width(st, tch, zhi)
                return [(d_w, w)]

            def emit_sc(st):
                w = causal_width(st, tch)
                d_w = dead_width(st, tch, zhi)
                k = st - first_st  # delta = 128*k
                sc_hp = ps_sc.tile([128, 2, TCH], F32, tag="sc",
                                   name=f"sc{j}_{st}")
                p_hp = scr.tile([128, 2, TCH], BF16, tag="p", bufs=12,
                                name=f"p{j}_{st}")
                for c0, c1 in block_ranges(st):
                    for i, h in enumerate(heads):
                        hp = (h % 2) * 64
                        nc.tensor.matmul(
                            sc_hp[:, i, c0:c1],
                            kt_sb[j][hp:hp + DH, 128 * st:128 * (st + 1)],
                            qt_sb[j][hp:hp + DH,
                                     TCH * tch + c0:TCH * tch + c1],
                            start=True, stop=True)
                    nc.scalar.activation(
                        p_hp[:, :, c0:c1], sc_hp[:, :, c0:c1],
                        mybir.ActivationFunctionType.Exp, scale=EXP_SCALE)
                    # one mask op covers BOTH heads ([128, 2, w] tiles and
                    # mt rows are head-pair adjacent) -> half the op count
                    # and half the chain latency
                    if k <= 3:
                        # diagonal block: causal zeroing on [128k, w)
                        d0 = 128 * k
                        v0, v1 = max(c0, d0), min(c1, w)
                        if v1 > v0:
                            nc.gpsimd.tensor_mul(
                                p_hp[:, :, v0:v1], p_hp[:, :, v0:v1],
                                c01_sb[:, k:k + 1, v0 - d0:v1 - d0]
                                .broadcast_to([128, 2, v1 - v0]))
                    if k in mt_offs:
                        # span mask: p *= mt (host-precomputed clamp);
                        # ramp bands are wide -> DVE (f16 2x), not gpsimd
                        off, mt_d, m_w = mt_offs[k]
                        v0, v1 = max(c0, mt_d), min(c1, m_w)
                        if v1 > v0:
                            nc.vector.tensor_mul(
                                p_hp[:, :, v0:v1], p_hp[:, :, v0:v1],
                                mt_sb[:, 2 * j:2 * j + 2,
                                      off + v0 - mt_d:off + v1 - mt_d])
                p_tiles[st] = p_hp

            def emit_av(st):
                av_first = ST - 1 if tch == 1 else first_st
                av_last = first_st if tch == 1 else ST - 1
                for c0, c1 in block_ranges(st):
                    for i, h in enumerate(heads):
                        nc.tensor.matmul(
                            pouts[i][:, c0:c1], v_aug[st][:, h, :],
                            p_tiles[st][:, i, c0:c1],
                            start=(st == av_first), stop=(st == av_last),
                            skip_group_check=True)

            # tch=1 pairs run largest-first: the pair ends on its smallest
            # exp, so the next pair's score banks are freed ~3x sooner at
            # the handoff (tch=0 keeps ascending; v-prefetch requires it and
            # its wide late blocks have no filler cover early)
            desc = tch == 1
            order = (list(range(ST - 1, first_st - 1, -1)) if desc
                     else list(range(first_st, ST)))
            fin_early, fin_late = None, None
            if prev_fin is not None:
                if fin_idx == 'split':
                    fin_early, fin_late = prev_fin.split()
                elif fin_idx == 0:
                    fin_early = prev_fin
                else:
                    fin_late = prev_fin
            emit_sc(order[0])
            if fin_early is not None:
                fin_early()
            for idx in range(1, len(order)):
                st = order[idx]
                if v_prefetch and st + 3 < ST and v_aug[st + 3] is None:
                    emit_v(st + 3, pool=ps_fill)
                if fillers:
                    fillers.pop(0)()
                emit_sc(st)
                # deferred finalize part: must precede this pair's first
                # attn@V (idx==2), which reuses the previous pout banks
                if idx == 2 and fin_late is not None:
                    fin_late()
                if idx >= 2:
                    emit_av(order[idx - 2])
            if fillers:
                fillers.pop(0)()
            emit_av(order[-2])
            emit_av(order[-1])

            def div_chunk(rws, c0, c1):
                for i, h in enumerate(heads):
                    hp = (h % 2) * 64
                    nc.vector.scalar_tensor_tensor(
                        out_pair[j][tch][hp:hp + DH, c0:c1],
                        pouts[i][0:DH, c0:c1], DESCALE, rws[i][:, c0:c1],
                        op0=mybir.AluOpType.mult,
                        op1=mybir.AluOpType.mult)

            def recips():
                # rows 0:64 numerator (scaled SX*SW); rows 64:128 denominator
                # W (unscaled); the division multiplier folds the V descale
                rws = []
                for i, h in enumerate(heads):
                    rw = scr.tile([DH, TCH], F32, tag="rw", bufs=8,
                                  name=f"rw{h}")
                    with nc.allow_low_precision(reason="denom recip bf16"):
                        nc.vector.reciprocal(rw[:], pouts[i][DH:2 * DH, :])
                    rws.append(rw)
                return rws

            def finalize():
                rws = recips()
                chunks = (0, 256, TCH) if split_fin else (0, TCH)
                for c0, c1 in zip(chunks[:-1], chunks[1:]):
                    div_chunk(rws, c0, c1)

            def finalize_split():
                # (early, late): early covers cols 0:256 (all the next
                # pair's y-filler reads for tt 0/1); late defers the rest
                st_ = {}

                def early():
                    st_['rws'] = recips()
                    div_chunk(st_['rws'], 0, 256)

                def late():
                    div_chunk(st_['rws'], 256, TCH)
                return early, late

            finalize.split = finalize_split
            return finalize

        copy_rot = [0]

        def y_group(tt, nch, pool, engines=("act", "pool"), yo=None):
            """One output-projection psum group; DMA fires unless yo is a
            shared per-tt tile whose DMA the caller batches."""
            tch = tt // 4
            toff = 128 * tt - TCH * tch
            yps = pool.tile([128, TCH], F32, tag="pj", name=f"y{tt}_{nch}")
            for j in range(4):
                nc.tensor.matmul(
                    yps[:],
                    out_pair[j][tch][:, toff:toff + 128],
                    wo[:, TCH * nch:TCH * (nch + 1), j],
                    start=(j == 0), stop=(j == 3))
            own_dma = yo is None
            if own_dma:
                yo_sl = ysb.tile([128, TCH], F16, tag="y", bufs=8)
            else:
                yo_sl = yo[:, TCH * nch:TCH * (nch + 1)]
            eng = engines[copy_rot[0] % len(engines)]
            copy_rot[0] += 1
            if eng == "dve":
                nc.vector.tensor_copy(yo_sl[:] if own_dma else yo_sl, yps[:])
            else:
                nc.scalar.copy(yo_sl[:] if own_dma else yo_sl, yps[:])
            if own_dma:
                nc.sync.dma_start(
                    yp_d[128 * tt:128 * (tt + 1), TCH * nch:TCH * (nch + 1)],
                    yo_sl[:])

        def y_fillers(tts):
            fs = []
            pend = [None]

            def make(tt, nch):
                def f():
                    tch0 = tt // 4
                    toff = 128 * tt - TCH * tch0
                    yps = ps_fill.tile([128, TCH], F32, tag="pj",
                                       name=f"y{tt}_{nch}")
                    for j in range(4):
                        nc.tensor.matmul(
                            yps[:],
                            out_pair[j][tch0][:, toff:toff + 128],
                            wo[:, TCH * nch:TCH * (nch + 1), j],
                            start=(j == 0), stop=(j == 3))

                    def copy_dma():
                        yo = ysb.tile([128, TCH], F16, tag="y", bufs=8)
                        nc.vector.tensor_copy(yo[:], yps[:])
                        nc.sync.dma_start(
                            yp_d[128 * tt:128 * (tt + 1),
                                 TCH * nch:TCH * (nch + 1)], yo[:])
                    prev, pend[0] = pend[0], copy_dma
                    if prev is not None:
                        prev()
                return f

            for tt in tts:
                for nch in range(NT):
                    fs.append(make(tt, nch))

            def flush():
                if pend[0] is not None:
                    pend[0]()
                    pend[0] = None
            fs.append(flush)
            return fs

        kf = proj_fillers([(kt_sb, WK, 1, 0), (kt_sb, WK, 1, 1)])
        f0 = [lambda: emit_v(4, pool=ps_fill), kf[0],
              lambda: emit_v(5, pool=ps_fill), kf[1],
              lambda: emit_v(6, pool=ps_fill), kf[2],
              lambda: emit_v(7, pool=ps_fill)]
        f1 = proj_fillers([(qt_sb, WQ, 2, 1), (kt_sb, WK, 2, 0),
                           (kt_sb, WK, 2, 1)])
        f2 = proj_fillers([(qt_sb, WQ, 3, 0), (qt_sb, WQ, 3, 1),
                           (kt_sb, WK, 3, 0)])
        f3 = proj_fillers([(kt_sb, WK, 3, 1)])
        fin = attn_pair(0, 0, fillers=f0)
        fin = attn_pair(0, 1, fillers=f1, prev_fin=fin, fin_idx=2)
        fin = attn_pair(0, 2, fillers=f2, prev_fin=fin, fin_idx=2)
        fin = attn_pair(0, 3, fillers=f3, prev_fin=fin, fin_idx=2)
        fin = attn_pair(1, 0, fillers=y_fillers([0]), prev_fin=fin,
                        fin_idx='split')
        fin = attn_pair(1, 1, fillers=y_fillers([1]), prev_fin=fin, fin_idx=2)
        fin = attn_pair(1, 2, fillers=y_fillers([2]), prev_fin=fin, fin_idx=2)
        fin = attn_pair(1, 3, fillers=y_fillers([3]), prev_fin=fin, fin_idx=2,
                        split_fin=True)
        fin()
        attn_ctx.close()
        with tc.tile_pool(name="ps_tail", bufs=4, space="PSUM") as ps_tail:
            # per-tt batched DMAs: 4 issues total, each overlapping the
            # remaining matmuls; the last chain is copy + one issue + xfer
            for tt in range(4, 7):
                yo = ysb.tile([128, D], F16, tag="yb", bufs=3)
                for nch in range(NT):
                    y_group(tt, nch, ps_tail,
                            engines=("act", "dve"), yo=yo)
                nc.sync.dma_start(yp_d[128 * tt:128 * (tt + 1), :], yo[:])
            for nch in range(NT):
                y_group(7, nch, ps_tail, engines=("act", "dve"))

    nc.compile()
    _NC_CACHE[key] = nc
    return nc


def _split_pack(a, scale):
    """[D, F] f32 -> (hi, lo) [128, DT, F] fp8e4 mega-tiles."""
    Dd, F = a.shape
    s = (a * scale).astype(np.float32)
    hi = s.astype(E4NP)
    lo = (s - hi.astype(np.float32)).astype(E4NP)
    def pack(m):
        return np.ascontiguousarray(
            m.reshape(Dd // 128, 128, F).transpose(1, 0, 2))
    return pack(hi), pack(lo)


def _pack_dtiles(w):
    """[D, F] -> [128, F, DT] mega-tile (partition, inner, d-tile)."""
    Dd, F = w.shape
    return np.ascontiguousarray(
        w.reshape(Dd // 128, 128, F).transpose(1, 2, 0))


def _prep_core_inputs(x, Wq, Wk, Wv, Wo, z, zlo, zhi, skip384):
    bf = ml_dtypes.bfloat16
    c01 = _make_c01()
    w_splits = []
    for W in (Wq, Wk, Wv):
        w_splits.append([
            _split_pack(np.ascontiguousarray(W[E * g:E * (g + 1), :].T), SW)
            for g in range(2)])
    wo_packs = [
        _pack_dtiles(np.ascontiguousarray(Wo[:, E * g:E * (g + 1)].T)).astype(bf)
        for g in range(2)]
    x_splits = [_split_pack(np.ascontiguousarray(x[b].T), SX)
                for b in range(B)]
    in_maps = []
    for c in range(N_CORES):
        b, g = c // 2, c % 2
        in_maps.append({
            "c01": c01,
            "xh8": x_splits[b][0],
            "xl8": x_splits[b][1],
            "Wqh8": w_splits[0][g][0], "Wql8": w_splits[0][g][1],
            "Wkh8": w_splits[1][g][0], "Wkl8": w_splits[1][g][1],
            "Wvh8": w_splits[2][g][0], "Wvl8": w_splits[2][g][1],
            "Wo8": wo_packs[g],
            "mt8": _make_mt(z[b, HC * g:HC * (g + 1)], zlo, zhi, skip384),
        })
    return in_maps


def _make_c01():
    sp = np.arange(128, dtype=np.float32)[:, None]
    jp = np.arange(128, dtype=np.float32)[None, :]
    m = (sp - jp >= 0).astype(np.float16)
    return np.ascontiguousarray(
        np.broadcast_to(m[:, None, :], (128, 4, 128))).astype(np.float16)


def _make_mt(z_h, zlo, zhi, skip384):
    """Per-head packed span-ramp tiles [128, HC, mt_cols] f16."""
    mt_offs, mt_cols = _mt_layout(zlo, zhi, skip384)
    out = np.zeros((128, HC, max(1, mt_cols)), np.float16)
    sp = np.arange(128, dtype=np.float32)[:, None]
    for k, (off, d_w, m_w) in mt_offs.items():
        tp = np.arange(d_w, m_w, dtype=np.float32)[None, :]
        d = 128.0 * k + sp - tp
        for h in range(HC):
            ramp = np.clip((R + z_h[h] - d) / R, 0.0, 1.0)
            ramp = np.where(d < 0, 0.0, ramp)  # causal-invalid -> 0
            out[:, h, off:off + m_w - d_w] = ramp.astype(np.float16)
    return out


def _nc_params(x, Wspan, bspan):
    """Span bounds from host-exact z; specializes mask widths per call."""
    x = np.asarray(x, np.float32)
    Wspan = np.asarray(Wspan, np.float32)
    bspan = np.asarray(bspan, np.float32)
    logits = x.mean(axis=1) @ Wspan.T + bspan
    z = T / (1.0 + np.exp(-logits))
    zlo = max(0, int(z.min() - 8.0) // 16 * 16)
    zhi = int(z.max() + 8.0) + 16
    # skipping the delta=384 ramp (dist in (z, 511]) perturbs <= (511-z)/R
    # of the weight on a sliver of columns; safe when z_min >= 491
    skip384 = bool(z.min() >= 491.0)
    return z, zlo, zhi, skip384


def kernel(x, Wq, Wk, Wv, Wo, bo, Wspan, bspan):
    x = np.asarray(x, np.float32)
    Wq = np.asarray(Wq, np.float32)
    Wk = np.asarray(Wk, np.float32)
    Wv = np.asarray(Wv, np.float32)
    Wo = np.asarray(Wo, np.float32)
    bo = np.asarray(bo, np.float32)
    Wspan = np.asarray(Wspan, np.float32)
    bspan = np.asarray(bspan, np.float32)

    z, zlo, zhi, skip384 = _nc_params(x, Wspan, bspan)
    nc = build_nc(zlo, zhi, skip384)
    in_maps = _prep_core_inputs(x, Wq, Wk, Wv, Wo, z, zlo, zhi, skip384)
    res = run_bass_kernel_spmd(nc, in_maps, core_ids=list(range(N_CORES)))
    y = np.empty((B, T, D), np.float32)
    for b in range(B):
        y[b] = (res.results[2 * b]["yp"].astype(np.float32)
                + res.results[2 * b + 1]["yp"].astype(np.float32) + bo)
    return y


# revision 95
# speedup vs baseline: 1.0006x; 1.0006x over previous
"""AdaptiveSpanAttention Trainium2 kernel (8 NeuronCores).

Sharding: core c -> (batch b = c//2, head-group g = c%2).
Each core computes, for its batch and its 8 heads:
  Q/K/V projections, anti-causal (j>=i) attention with adaptive-span
  mask, renormalization, and a partial output projection
  y_part = Out_g @ Wo[:, e_slice].T  (contraction over its 512 channels).
Host combines: y[b] = y_part[2b] + y_part[2b+1] + bo.

Q/K/V projections run as fp8e4 DoubleRow matmuls with a hi/lo split
(x = xh + xl, W = wh + wl after power-of-2 pre-scaling; the three
products xh*wh, xl*wh, xh*wl share one scale so they accumulate into a
single PSUM group at 0.75x the bf16 instruction cost). The descale
factors fold into the exp scale (scores) and the finalize multiplier
(V path), so no extra vector work is added. Attention itself stays
bf16 (64-deep score contraction cannot pack DoubleRow slots).

The span network z = T*sigmoid(mean_t x @ Wspan + bspan) is computed
on host (it already ran there for mask-width specialization); the
kernel receives pre-clamped span-ramp tiles mt[h,k] and multiplies
them in directly.

Schedule notes:
- host packs x / weights into hi/lo fp8 mega-tiles ([128, DT, T] /
  [128, DT, E]) so each tensor is 1-2 large contiguous DMAs, ordered so
  V -> Q -> K groups start as their operands land.
- attention is software-pipelined: scores for block st+1 are issued
  before attn@V of block st so the exp+mask chain hides under PE work;
  one mask multiply covers both heads of a pair (causal on Pool, wide
  span ramps on DVE).
- span bounds specialized per call from host-computed z: mask ops only
  on the ramp band, fully-masked score columns skipped.
"""
import sys

sys.path.insert(0, "/opt/trn_rl_repo")

from contextlib import ExitStack

import ml_dtypes
import numpy as np

import concourse.bass as bass
import concourse.tile as tile
from concourse import bacc, mybir
from concourse.bass_utils import run_bass_kernel_spmd

BF16 = mybir.dt.bfloat16
F16 = mybir.dt.float16
F32 = mybir.dt.float32
FP8 = mybir.dt.float8e4
DR = mybir.MatmulPerfMode.DoubleRow
E4NP = ml_dtypes.float8_e4m3fn

B, T, D, H = 4, 1024, 1024, 16
DH = 64          # head dim
R = 256.0
HC = 8           # heads per core
E = 512          # channels per core (HC * DH)
N_CORES = 8
TCH = 512        # t-chunk width (PSUM f32 free-dim limit)
NT = T // TCH    # 2 t-chunks
ST = T // 128    # 8 s-tiles
DT = D // 128    # 8 d-tiles

SX = 32.0        # x pre-scale (|x| < 6 -> |x*SX| < 200, e4m3 max 448)
SW = 1024.0      # W pre-scale (|W| < 0.2)
DESCALE = 1.0 / (SX * SW)

_NC_CACHE = {}


def causal_width(st, tch):
    """Valid query-column width of block (s_tile=st, t_chunk=tch)."""
    delta = 128 * st - 512 * tch
    return max(0, min(TCH, delta + 128))


def span_width(st, tch, zlo):
    """Columns [0, m_w) where the span mask can differ from 1 (z >= zlo)."""
    delta = 128 * st - 512 * tch
    w = causal_width(st, tch)
    return max(0, min(w, delta + 127 - int(zlo)))


def dead_width(st, tch, zhi):
    """Columns [0, d_w) where the span mask is identically 0 (z <= zhi)."""
    delta = 128 * st - 512 * tch
    w = causal_width(st, tch)
    return max(0, min(w, int(delta - R - zhi) // 16 * 16))


def _mt_layout(zlo, zhi, skip384):
    """(offsets dict {(k): (off, d_w, m_w)}, total cols) of the packed
    per-head span-ramp table; per-head stride is the total."""
    offs = {}
    off = 0
    for k in range(ST):
        m_w = span_width(k, 0, zlo)
        if m_w <= 0 or (k == 3 and skip384):
            continue
        d_w = dead_width(k, 0, zhi)
        if m_w <= d_w:
            continue
        offs[k] = (off, d_w, m_w)
        off += m_w - d_w
    return offs, off


def build_nc(zlo, zhi, skip384):
    key = (zlo, zhi, skip384)
    if key in _NC_CACHE:
        return _NC_CACHE[key]
    nc = bacc.Bacc("TRN2", target_bir_lowering=False, debug=False, num_devices=1)

    mt_offs, mt_cols = _mt_layout(zlo, zhi, skip384)

    # ---- DRAM parameters (per-core shards prepared on host) ----
    # hi/lo fp8 split tensors in [128, DT, free] mega-tile layout; DoubleRow
    # slots pair adjacent d-tiles of one product type (classic [P, 2, M]
    # stationary APs that walrus accepts; all slices DMA-contiguous)
    xh_d = nc.declare_dram_parameter("xh8", [128, DT, T], FP8, isOutput=False)
    xl_d = nc.declare_dram_parameter("xl8", [128, DT, T], FP8, isOutput=False)
    Wqh_d = nc.declare_dram_parameter("Wqh8", [128, DT, E], FP8, isOutput=False)
    Wql_d = nc.declare_dram_parameter("Wql8", [128, DT, E], FP8, isOutput=False)
    Wkh_d = nc.declare_dram_parameter("Wkh8", [128, DT, E], FP8, isOutput=False)
    Wkl_d = nc.declare_dram_parameter("Wkl8", [128, DT, E], FP8, isOutput=False)
    Wvh_d = nc.declare_dram_parameter("Wvh8", [128, DT, E], FP8, isOutput=False)
    Wvl_d = nc.declare_dram_parameter("Wvl8", [128, DT, E], FP8, isOutput=False)
    Wo_d = nc.declare_dram_parameter("Wo8", [128, D, 4], BF16, isOutput=False)
    # packed span-ramp tiles: mt8[p, h, off_k + j] = clamp((R+z_h-d)/R, 0, 1)
    # for d = 128k + p - (d_w + j); 0 where causal-invalid
    mt_d = nc.declare_dram_parameter("mt8", [128, HC, max(1, mt_cols)],
                                     F16, isOutput=False)
    # c01[s', k, j] = 1.0 if s' >= j else 0.0  (causal 0/1 for t' = 128k + j)
    c01_d = nc.declare_dram_parameter("c01", [128, 4, 128], F16, isOutput=False)
    yp_d = nc.declare_dram_parameter("yp", [T, D], F16, isOutput=True)

    with tile.TileContext(nc) as tc, ExitStack() as ctx:
        # ---------------- pools ----------------
        consts = ctx.enter_context(tc.tile_pool(name="consts", bufs=1))
        xp = ctx.enter_context(tc.tile_pool(name="xp", bufs=1))
        wp = ctx.enter_context(tc.tile_pool(name="wp", bufs=1))
        qkp = ctx.enter_context(tc.tile_pool(name="qkp", bufs=1))
        vp = ctx.enter_context(tc.tile_pool(name="vp", bufs=1))
        outp = ctx.enter_context(tc.tile_pool(name="outp", bufs=1))
        scr = ctx.enter_context(tc.tile_pool(name="scr", bufs=3))
        ysb = ctx.enter_context(tc.tile_pool(name="ysb", bufs=6))

        lead_ctx = ExitStack()
        ps_lead = lead_ctx.enter_context(
            tc.tile_pool(name="ps_lead", bufs=8, space="PSUM"))

        # ---------------- loads (few large DMAs) ----------------
        xh = xp.tile([128, DT, T], FP8, name="xh8")
        xl = xp.tile([128, DT, T], FP8, name="xl8")
        wqh = wp.tile([128, DT, E], FP8, name="wqh8")
        wql = wp.tile([128, DT, E], FP8, name="wql8")
        wkh = wp.tile([128, DT, E], FP8, name="wkh8")
        wkl = wp.tile([128, DT, E], FP8, name="wkl8")
        wvh = wp.tile([128, DT, E], FP8, name="wvh8")
        wvl = wp.tile([128, DT, E], FP8, name="wvl8")
        wo = wp.tile([128, D, 4], BF16, name="wo8")
        # DMA order tuned to the lead emission: V groups first (largest PE
        # block), then Q t-chunks, then K; mt/c01/wo only feed the
        # attention phase and stream last
        # first two tensors stream in dt-halves so the first V matmuls
        # start ~1.5us sooner
        nc.sync.dma_start(xh[:, 0:4, 0:TCH], xh_d[:, 0:4, 0:TCH])
        nc.sync.dma_start(wvh[:, 0:4, :], Wvh_d[:, 0:4, :])
        nc.sync.dma_start(xh[:, 4:DT, 0:TCH], xh_d[:, 4:DT, 0:TCH])
        nc.sync.dma_start(wvh[:, 4:DT, :], Wvh_d[:, 4:DT, :])
        nc.sync.dma_start(xl[:, :, 0:TCH], xl_d[:, :, 0:TCH])
        nc.sync.dma_start(wvl[:], Wvl_d[:, :, :])
        nc.sync.dma_start(wqh[:], Wqh_d[:, :, :])
        nc.sync.dma_start(wql[:], Wql_d[:, :, :])
        nc.sync.dma_start(xh[:, :, TCH:T], xh_d[:, :, TCH:T])
        nc.sync.dma_start(wkh[:], Wkh_d[:, :, :])
        nc.sync.dma_start(xl[:, :, TCH:T], xl_d[:, :, TCH:T])
        nc.sync.dma_start(wkl[:], Wkl_d[:, :, :])
        c01_sb = consts.tile([128, 4, 128], F16, tag="c01")
        nc.sync.dma_start(c01_sb[:], c01_d[:, :, :])
        # mt sliced per head-pair: with attention starting ~21.5us, pair
        # (0,0)'s first ramp block would otherwise race the full-table DMA
        mt_sb = consts.tile([128, HC, max(1, mt_cols)], F16, tag="mt8")
        nc.sync.dma_start(mt_sb[:, 0:2, :], mt_d[:, 0:2, :])
        nc.sync.dma_start(mt_sb[:, 2:4, :], mt_d[:, 2:4, :])
        nc.sync.dma_start(wo[:], Wo_d[:, :, :])
        nc.sync.dma_start(mt_sb[:, 4:6, :], mt_d[:, 4:6, :])
        nc.sync.dma_start(mt_sb[:, 6:8, :], mt_d[:, 6:8, :])

        # ---------------- Q/K projections (transposed layout) ----------------
        # QT[e, t] = sum_d W[d, e] * xT[d, t] in fp8 DoubleRow 3-term:
        # per dtile (wh)x(xh,xl), per dtile-pair (wl_d,wl_d+1)x(xh_d,xh_d+1)
        qt_sb = [qkp.tile([128, T], BF16, tag="qt", name=f"qt{i}", bufs=4)
                 for i in range(4)]
        kt_sb = [qkp.tile([128, T], BF16, tag="kt", name=f"kt{i}", bufs=4)
                 for i in range(4)]

        def proj_mms(ps, w_hl, et, t0, t1):
            w_hi, w_lo = w_hl
            eb = 128 * et
            w_cols = t1 - t0
            for i, (wt, xt) in enumerate(
                    ((w_hi, xh), (w_hi, xl), (w_lo, xh))):
                for dt_i in range(0, DT, 2):
                    nc.tensor.matmul(
                        ps[:, 0:w_cols],
                        wt[:, dt_i:dt_i + 2, eb:eb + 128],
                        xt[:, dt_i:dt_i + 2, t0:t1],
                        start=(i == 0 and dt_i == 0),
                        stop=(i == 2 and dt_i == DT - 2), perf_mode=DR)

        def emit_proj(dst, w8, et, t0, t1, copy_eng="act", pool=None):
            pool = pool or ps_lead
            ps = pool.tile([128, TCH], F32, tag="pj", name=f"pj{et}_{t0}",
                           padded_shape=[128, TCH])
            proj_mms(ps, w8, et, t0, t1)
            if copy_eng == "act":
                nc.scalar.copy(dst[et][:, t0:t1], ps[:, 0:t1 - t0])
            else:
                nc.vector.tensor_copy(dst[et][:, t0:t1], ps[:, 0:t1 - t0])

        WQ = (wqh, wql)
        WK = (wkh, wkl)

        # ---------------- V (natural layout, ones-augmented) ----------------
        # v_aug[st][p, h, 0:64] = V_raw[128*st+p, 64h+j] (scaled SX*SW);
        # v_aug[st][p, h, 64:128] = 1 (exact denominator rows; the V descale
        # folds into the finalize multiplier)
        v_aug = [None] * ST

        def emit_v(st, pool=None, copy_eng="dve"):
            pool = pool or ps_lead
            va = vp.tile([128, HC, 2 * DH], BF16, tag="vaug", bufs=ST,
                         name=f"vaug{st}")
            nc.gpsimd.memset(va[:, :, DH:2 * DH], 1.0)
            ps = pool.tile([128, E], F32, tag="pj", name=f"pjv{st}")
            sb = 128 * st
            for i, (xt, wt) in enumerate(
                    ((xh, wvh), (xl, wvh), (xh, wvl))):
                for dt_i in range(0, DT, 2):
                    nc.tensor.matmul(
                        ps[:],
                        xt[:, dt_i:dt_i + 2, sb:sb + 128],
                        wt[:, dt_i:dt_i + 2, :],
                        start=(i == 0 and dt_i == 0),
                        stop=(i == 2 and dt_i == DT - 2), perf_mode=DR)
            ceng = nc.scalar.copy if copy_eng == "act" else nc.vector.tensor_copy
            ceng(va[:, :, 0:DH], ps[:].rearrange("p (h d) -> p h d", h=HC))
            v_aug[st] = va

        # lead emission ordered by DMA arrival: V st0-5 first (x + wv),
        # then Q chunks, then K et0 only -- K et1, V6, V7 move into pair
        # (0,0)'s fillers so attention starts ~5us earlier and that work
        # lands in the attention phase's stall pockets
        for st in range(4):
            emit_v(st, copy_eng=("dve" if st < 2 else "act"))
        emit_proj(qt_sb, WQ, 0, 0, TCH)
        emit_proj(qt_sb, WQ, 1, 0, TCH)
        emit_proj(qt_sb, WQ, 0, TCH, T)
        emit_proj(qt_sb, WQ, 1, TCH, T)
        # K et0 copies go on DVE: they land right at attention start and
        # must not queue ahead of the first exp ops on ACT
        for tch in range(NT):
            emit_proj(kt_sb, WK, 0, TCH * tch, TCH * (tch + 1),
                      copy_eng="dve")

        # spare projection work at the lead tail keeps PE busy while the
        # first score block's psum bank clears its lead-phase WAR; two
        # half-width groups make the last copy (the WAR reader) short
        ps_sp1 = ps_lead.tile([128, 256], F32, tag="pj", name="pjsp1",
                              padded_shape=[128, TCH])
        proj_mms(ps_sp1, WQ, 2, 0, 256)
        nc.scalar.copy(qt_sb[2][:, 0:256], ps_sp1[:, 0:256])
        ps_sp2 = ps_lead.tile([128, 256], F32, tag="pj", name="pjsp2",
                              padded_shape=[128, TCH])
        proj_mms(ps_sp2, WQ, 2, 256, TCH)
        nc.vector.tensor_copy(qt_sb[2][:, 256:TCH], ps_sp2[:, 0:256])

        lead_ctx.close()
        attn_ctx = ExitStack()
        ps_sc = attn_ctx.enter_context(
            tc.tile_pool(name="ps_sc", bufs=2, space="PSUM"))
        ps_out = attn_ctx.enter_context(
            tc.tile_pool(name="ps_out", bufs=2, space="PSUM"))
        ps_fill = attn_ctx.enter_context(
            tc.tile_pool(name="ps_fill", bufs=2, space="PSUM"))

        def proj_fillers(specs):
            """Filler closures whose psum->SBUF copy is deferred one slot so
            it queues behind the current block's exp/mask, not ahead."""
            fs = []
            pend = [None]

            def make(dst, w8, et, tch):
                def f():
                    ps = ps_fill.tile([128, TCH], F32, tag="pj",
                                      name=f"pjf{et}_{tch}",
                                      padded_shape=[128, TCH])
                    proj_mms(ps, w8, et, TCH * tch, TCH * (tch + 1))
                    prev, pend[0] = pend[0], (
                        lambda: nc.vector.tensor_copy(
                            dst[et][:, TCH * tch:TCH * (tch + 1)], ps[:]))
                    if prev is not None:
                        prev()
                return f

            for dst, w8, et, tch in specs:
                fs.append(make(dst, w8, et, tch))

            def flush():
                if pend[0] is not None:
                    pend[0]()
                    pend[0] = None
            fs.append(flush)
            return fs

        # ---------------- attention ----------------
        # out_pair[j][tch] holds heads 2j (parts 0:64) and 2j+1 (parts 64:128)
        out_pair = [[outp.tile([128, TCH], BF16, tag="out", bufs=8,
                               name=f"op{j}_{c}") for c in range(NT)]
                    for j in range(4)]

        # exp absorbs the Q/K descales: p = exp(s_raw / (8 * (SX*SW)^2))
        EXP_SCALE = 1.0 / (8.0 * (SX * SW) ** 2)

        def attn_pair(tch, j, v_prefetch=False, fillers=(), prev_fin=None,
                      split_fin=False, split_exp=False, fin_idx=0):
            """Attention for head pair (2j, 2j+1); both share et=j.

            Scores for the two heads go into one 2-bank psum pair-tile so a
            single exp covers both. Scores run one block ahead of attn@V so
            the exp+mask chain hides under PE work. The previous pair's
            out-division (prev_fin) is emitted after this pair's first score
            block so it does not wedge ahead of this pair's mask ops in the
            DVE queue. Returns this pair's finalize closure.
            """
            first_st = 4 * tch
            heads = (2 * j, 2 * j + 1)
            pouts = [ps_out.tile([128, TCH], F32, tag="pout",
                                 name=f"pout{h}_{tch}") for h in heads]
            fillers = list(fillers)
            p_tiles = {}

            def block_ranges(st):
                w = causal_width(st, tch)
                d_w = dead_width(st, tch, zhi)
                return [(d_w, w)]

            def emit_sc(st):
                w = causal_width(st, tch)
                d_w = dead_width(st, tch, zhi)
                k = st - first_st  # delta = 128*k
                sc_hp = ps_sc.tile([128, 2, TCH], F32, tag="sc",
                                   name=f"sc{j}_{st}")
                p_hp = scr.tile([128, 2, TCH], BF16, tag="p", bufs=12,
                                name=f"p{j}_{st}")
                for c0, c1 in block_ranges(st):
                    for i, h in enumerate(heads):
                        hp = (h % 2) * 64
                        nc.tensor.matmul(
                            sc_hp[:, i, c0:c1],
                            kt_sb[j][hp:hp + DH, 128 * st:128 * (st + 1)],
                            qt_sb[j][hp:hp + DH,
                                     TCH * tch + c0:TCH * tch + c1],
                            start=True, stop=True)
                    nc.scalar.activation(
                        p_hp[:, :, c0:c1], sc_hp[:, :, c0:c1],
                        mybir.ActivationFunctionType.Exp, scale=EXP_SCALE)
                    # one mask op covers BOTH heads ([128, 2, w] tiles and
                    # mt rows are head-pair adjacent) -> half the op count
                    # and half the chain latency
                    if k <= 3:
                        # diagonal block: causal zeroing on [128k, w)
                        d0 = 128 * k
                        v0, v1 = max(c0, d0), min(c1, w)
                        if v1 > v0:
                            nc.gpsimd.tensor_mul(
                                p_hp[:, :, v0:v1], p_hp[:, :, v0:v1],
                                c01_sb[:, k:k + 1, v0 - d0:v1 - d0]
                                .broadcast_to([128, 2, v1 - v0]))
                    if k in mt_offs:
                        # span mask: p *= mt (host-precomputed clamp);
                        # ramp bands are wide -> DVE (f16 2x), not gpsimd
                        off, mt_d, m_w = mt_offs[k]
                        v0, v1 = max(c0, mt_d), min(c1, m_w)
                        if v1 > v0:
                            nc.vector.tensor_mul(
                                p_hp[:, :, v0:v1], p_hp[:, :, v0:v1],
                                mt_sb[:, 2 * j:2 * j + 2,
                                      off + v0 - mt_d:off + v1 - mt_d])
                p_tiles[st] = p_hp

            def emit_av(st):
                av_first = ST - 1 if tch == 1 else first_st
                av_last = first_st if tch == 1 else ST - 1
                for c0, c1 in block_ranges(st):
                    for i, h in enumerate(heads):
                        nc.tensor.matmul(
                            pouts[i][:, c0:c1], v_aug[st][:, h, :],
                            p_tiles[st][:, i, c0:c1],
                            start=(st == av_first), stop=(st == av_last),
                            skip_group_check=True)

            # tch=1 pairs run largest-first: the pair ends on its smallest
            # exp, so the next pair's score banks are freed ~3x sooner at
            # the handoff (tch=0 keeps ascending; v-prefetch requires it and
            # its wide late blocks have no filler cover early)
            desc = tch == 1
            order = (list(range(ST - 1, first_st - 1, -1)) if desc
                     else list(range(first_st, ST)))
            fin_early, fin_late = None, None
            if prev_fin is not None:
                if fin_idx == 'split':
                    fin_early, fin_late = prev_fin.split()
                elif fin_idx == 0:
                    fin_early = prev_fin
                else:
                    fin_late = prev_fin
            emit_sc(order[0])
            if fin_early is not None:
                fin_early()
            for idx in range(1, len(order)):
                st = order[idx]
                if v_prefetch and st + 3 < ST and v_aug[st + 3] is None:
                    emit_v(st + 3, pool=ps_fill)
                if fillers:
                    fillers.pop(0)()
                emit_sc(st)
                # deferred finalize part: must precede this pair's first
                # attn@V (idx==2), which reuses the previous pout banks
                if idx == 2 and fin_late is not None:
                    fin_late()
                if idx >= 2:
                    emit_av(order[idx - 2])
            if fillers:
                fillers.pop(0)()
            emit_av(order[-2])
            emit_av(order[-1])

            def div_chunk(rws, c0, c1):
                for i, h in enumerate(heads):
                    hp = (h % 2) * 64
                    nc.vector.scalar_tensor_tensor(
                        out_pair[j][tch][hp:hp + DH, c0:c1],
                        pouts[i][0:DH, c0:c1], DESCALE, rws[i][:, c0:c1],
                        op0=mybir.AluOpType.mult,
                        op1=mybir.AluOpType.mult)

            def recips():
                # rows 0:64 numerator (scaled SX*SW); rows 64:128 denominator
                # W (unscaled); the division multiplier folds the V descale
                rws = []
                for i, h in enumerate(heads):
                    rw = scr.tile([DH, TCH], F32, tag="rw", bufs=8,
                                  name=f"rw{h}")
                    with nc.allow_low_precision(reason="denom recip bf16"):
                        nc.vector.reciprocal(rw[:], pouts[i][DH:2 * DH, :])
                    rws.append(rw)
                return rws

            def finalize():
                rws = recips()
                chunks = (0, 256, TCH) if split_fin else (0, TCH)
                for c0, c1 in zip(chunks[:-1], chunks[1:]):
                    div_chunk(rws, c0, c1)

            def finalize_split():
                # (early, late): early covers cols 0:256 (all the next
                # pair's y-filler reads for tt 0/1); late defers the rest
                st_ = {}

                def early():
                    st_['rws'] = recips()
                    div_chunk(st_['rws'], 0, 256)

                def late():
                    div_chunk(st_['rws'], 256, TCH)
                return early, late

            finalize.split = finalize_split
            return finalize

        copy_rot = [0]

        def y_group(tt, nch, pool, engines=("act", "pool"), yo=None):
            """One output-projection psum group; DMA fires unless yo is a
            shared per-tt tile whose DMA the caller batches."""
            tch = tt // 4
            toff = 128 * tt - TCH * tch
            yps = pool.tile([128, TCH], F32, tag="pj", name=f"y{tt}_{nch}")
            for j in range(4):
                nc.tensor.matmul(
                    yps[:],
                    out_pair[j][tch][:, toff:toff + 128],
                    wo[:, TCH * nch:TCH * (nch + 1), j],
                    start=(j == 0), stop=(j == 3))
            own_dma = yo is None
            if own_dma:
                yo_sl = ysb.tile([128, TCH], F16, tag="y", bufs=8)
            else:
                yo_sl = yo[:, TCH * nch:TCH * (nch + 1)]
            eng = engines[copy_rot[0] % len(engines)]
            copy_rot[0] += 1
            if eng == "dve":
                nc.vector.tensor_copy(yo_sl[:] if own_dma else yo_sl, yps[:])
            else:
                nc.scalar.copy(yo_sl[:] if own_dma else yo_sl, yps[:])
            if own_dma:
                nc.sync.dma_start(
                    yp_d[128 * tt:128 * (tt + 1), TCH * nch:TCH * (nch + 1)],
                    yo_sl[:])

        def y_fillers(tts):
            fs = []
            pend = [None]

            def make(tt, nch):
                def f():
                    tch0 = tt // 4
                    toff = 128 * tt - TCH * tch0
                    yps = ps_fill.tile([128, TCH], F32, tag="pj",
                                       name=f"y{tt}_{nch}")
                    for j in range(4):
                        nc.tensor.matmul(
                            yps[:],
                            out_pair[j][tch0][:, toff:toff + 128],
                            wo[:, TCH * nch:TCH * (nch + 1), j],
                            start=(j == 0), stop=(j == 3))

                    def copy_dma():
                        yo = ysb.tile([128, TCH], F16, tag="y", bufs=8)
                        nc.vector.tensor_copy(yo[:], yps[:])
                        nc.sync.dma_start(
                            yp_d[128 * tt:128 * (tt + 1),
                                 TCH * nch:TCH * (nch + 1)], yo[:])
                    prev, pend[0] = pend[0], copy_dma
                    if prev is not None:
                        prev()
                return f

            for tt in tts:
                for nch in range(NT):
                    fs.append(make(tt, nch))

            def flush():
                if pend[0] is not None:
                    pend[0]()
                    pend[0] = None
            fs.append(flush)
            return fs

        kf = proj_fillers([(kt_sb, WK, 1, 0), (kt_sb, WK, 1, 1)])
        f0 = [lambda: emit_v(4, pool=ps_fill), kf[0],
              lambda: emit_v(5, pool=ps_fill), kf[1],
              lambda: emit_v(6, pool=ps_fill), kf[2],
              lambda: emit_v(7, pool=ps_fill)]
        f1 = proj_fillers([(qt_sb, WQ, 2, 1), (kt_sb, WK, 2, 0),
                           (kt_sb, WK, 2, 1)])
        f2 = proj_fillers([(qt_sb, WQ, 3, 0), (qt_sb, WQ, 3, 1),
                           (kt_sb, WK, 3, 0)])
        f3 = proj_fillers([(kt_sb, WK, 3, 1)])
        fin = attn_pair(0, 0, fillers=f0)
        fin = attn_pair(0, 1, fillers=f1, prev_fin=fin, fin_idx=2)
        fin = attn_pair(0, 2, fillers=f2, prev_fin=fin, fin_idx=2)
        fin = attn_pair(0, 3, fillers=f3, prev_fin=fin, fin_idx=2)
        fin = attn_pair(1, 0, fillers=y_fillers([0]), prev_fin=fin,
                        fin_idx='split')
        fin = attn_pair(1, 1, fillers=y_fillers([1]), prev_fin=fin, fin_idx=2)
        fin = attn_pair(1, 2, fillers=y_fillers([2]), prev_fin=fin, fin_idx=2)
        fin = attn_pair(1, 3, fillers=y_fillers([3]), prev_fin=fin, fin_idx=2,
                        split_fin=True)
        fin()
        attn_ctx.close()
        with tc.tile_pool(name="ps_tail", bufs=4, space="PSUM") as ps_tail:
            # per-tt batched DMAs: 4 issues total, each overlapping the
            # remaining matmuls; the last chain is copy + one issue + xfer
            for tt in range(4, 7):
                yo = ysb.tile([128, D], F16, tag="yb", bufs=3)
                for nch in range(NT):
                    y_group(tt, nch, ps_tail,
                            engines=("act", "dve"), yo=yo)
                nc.sync.dma_start(yp_d[128 * tt:128 * (tt + 1), :], yo[:])
            for nch in range(NT):
                y_group(7, nch, ps_tail, engines=("act", "dve"))

    nc.compile()
    _NC_CACHE[key] = nc
    return nc


def _split_pack(a, scale):
    """[D, F] f32 -> (hi, lo) [128, DT, F] fp8e4 mega-tiles."""
    Dd, F = a.shape
    s = (a * scale).astype(np.float32)
    hi = s.astype(E4NP)
    lo = (s - hi.astype(np.float32)).astype(E4NP)
    def pack(m):
        return np.ascontiguousarray(
            m.reshape(Dd // 128, 128, F).transpose(1, 0, 2))
    return pack(hi), pack(lo)


def _pack_dtiles(w):
    """[D, F] -> [128, F, DT] mega-tile (partition, inner, d-tile)."""
    Dd, F = w.shape
    return np.ascontiguousarray(
        w.reshape(Dd // 128, 128, F).transpose(1, 2, 0))


def _prep_core_inputs(x, Wq, Wk, Wv, Wo, z, zlo, zhi, skip384):
    bf = ml_dtypes.bfloat16
    c01 = _make_c01()
    w_splits = []
    for W in (Wq, Wk, Wv):
        w_splits.append([
            _split_pack(np.ascontiguousarray(W[E * g:E * (g + 1), :].T), SW)
            for g in range(2)])
    wo_packs = [
        _pack_dtiles(np.ascontiguousarray(Wo[:, E * g:E * (g + 1)].T)).astype(bf)
        for g in range(2)]
    x_splits = [_split_pack(np.ascontiguousarray(x[b].T), SX)
                for b in range(B)]
    in_maps = []
    for c in range(N_CORES):
        b, g = c // 2, c % 2
        in_maps.append({
            "c01": c01,
            "xh8": x_splits[b][0],
            "xl8": x_splits[b][1],
            "Wqh8": w_splits[0][g][0], "Wql8": w_splits[0][g][1],
            "Wkh8": w_splits[1][g][0], "Wkl8": w_splits[1][g][1],
            "Wvh8": w_splits[2][g][0], "Wvl8": w_splits[2][g][1],
            "Wo8": wo_packs[g],
            "mt8": _make_mt(z[b, HC * g:HC * (g + 1)], zlo, zhi, skip384),
        })
    return in_maps


def _make_c01():
    sp = np.arange(128, dtype=np.float32)[:, None]
    jp = np.arange(128, dtype=np.float32)[None, :]
    m = (sp - jp >= 0).astype(np.float16)
    return np.ascontiguousarray(
        np.broadcast_to(m[:, None, :], (128, 4, 128))).astype(np.float16)


def _make_mt(z_h, zlo, zhi, skip384):
    """Per-head packed span-ramp tiles [128, HC, mt_cols] f16."""
    mt_offs, mt_cols = _mt_layout(zlo, zhi, skip384)
    out = np.zeros((128, HC, max(1, mt_cols)), np.float16)
    sp = np.arange(128, dtype=np.float32)[:, None]
    for k, (off, d_w, m_w) in mt_offs.items():
        tp = np.arange(d_w, m_w, dtype=np.float32)[None, :]
        d = 128.0 * k + sp - tp
        for h in range(HC):
            ramp = np.clip((R + z_h[h] - d) / R, 0.0, 1.0)
            ramp = np.where(d < 0, 0.0, ramp)  # causal-invalid -> 0
            out[:, h, off:off + m_w - d_w] = ramp.astype(np.float16)
    return out


def _nc_params(x, Wspan, bspan):
    """Span bounds from host-exact z; specializes mask widths per call."""
    x = np.asarray(x, np.float32)
    Wspan = np.asarray(Wspan, np.float32)
    bspan = np.asarray(bspan, np.float32)
    logits = x.mean(axis=1) @ Wspan.T + bspan
    z = T / (1.0 + np.exp(-logits))
    # exact bounds: the device uses host-built mask tables, so the widths
    # only need zlo <= z.min (ramp-band start) and zhi > z.max (dead cols)
    zlo = max(0, int(z.min()))
    zhi = int(z.max()) + 1
    # skipping the delta=384 ramp (dist in (z, 511]) perturbs <= (511-z)/R
    # of the weight on a sliver of columns; safe when z_min >= 491
    skip384 = bool(z.min() >= 491.0)
    return z, zlo, zhi, skip384


def kernel(x, Wq, Wk, Wv, Wo, bo, Wspan, bspan):
    x = np.asarray(x, np.float32)
    Wq = np.asarray(Wq, np.float32)
    Wk = np.asarray(Wk, np.float32)
    Wv = np.asarray(Wv, np.float32)
    Wo = np.asarray(Wo, np.float32)
    bo = np.asarray(bo, np.float32)
    Wspan = np.asarray(Wspan, np.float32)
    bspan = np.asarray(bspan, np.float32)

    z, zlo, zhi, skip384 = _nc_params(x, Wspan, bspan)
    nc = build_nc(zlo, zhi, skip384)
    in_maps = _prep_core_inputs(x, Wq, Wk, Wv, Wo, z, zlo, zhi, skip384)
    res = run_bass_kernel_spmd(nc, in_maps, core_ids=list(range(N_CORES)))
    y = np.empty((B, T, D), np.float32)
    for b in range(B):
        y[b] = (res.results[2 * b]["yp"].astype(np.float32)
                + res.results[2 * b + 1]["yp"].astype(np.float32) + bo)
    return y


# revision 96
# speedup vs baseline: 1.0011x; 1.0005x over previous
"""AdaptiveSpanAttention Trainium2 kernel (8 NeuronCores).

Sharding: core c -> (batch b = c//2, head-group g = c%2).
Each core computes, for its batch and its 8 heads:
  Q/K/V projections, anti-causal (j>=i) attention with adaptive-span
  mask, renormalization, and a partial output projection
  y_part = Out_g @ Wo[:, e_slice].T  (contraction over its 512 channels).
Host combines: y[b] = y_part[2b] + y_part[2b+1] + bo.

Q/K/V projections run as fp8e4 DoubleRow matmuls with a hi/lo split
(x = xh + xl, W = wh + wl after power-of-2 pre-scaling; the three
products xh*wh, xl*wh, xh*wl share one scale so they accumulate into a
single PSUM group at 0.75x the bf16 instruction cost). The descale
factors fold into the exp scale (scores) and the finalize multiplier
(V path), so no extra vector work is added. Attention itself stays
bf16 (64-deep score contraction cannot pack DoubleRow slots).

The span network z = T*sigmoid(mean_t x @ Wspan + bspan) is computed
on host (it already ran there for mask-width specialization); the
kernel receives pre-clamped span-ramp tiles mt[h,k] and multiplies
them in directly.

Schedule notes:
- host packs x / weights into hi/lo fp8 mega-tiles ([128, DT, T] /
  [128, DT, E]) so each tensor is 1-2 large contiguous DMAs, ordered so
  V -> Q -> K groups start as their operands land.
- attention is software-pipelined: scores for block st+1 are issued
  before attn@V of block st so the exp+mask chain hides under PE work;
  one mask multiply covers both heads of a pair (causal on Pool, wide
  span ramps on DVE).
- span bounds specialized per call from host-computed z: mask ops only
  on the ramp band, fully-masked score columns skipped.
"""
import sys

sys.path.insert(0, "/opt/trn_rl_repo")

from contextlib import ExitStack

import ml_dtypes
import numpy as np

import concourse.bass as bass
import concourse.tile as tile
from concourse import bacc, mybir
from concourse.bass_utils import run_bass_kernel_spmd

BF16 = mybir.dt.bfloat16
F16 = mybir.dt.float16
F32 = mybir.dt.float32
FP8 = mybir.dt.float8e4
DR = mybir.MatmulPerfMode.DoubleRow
E4NP = ml_dtypes.float8_e4m3fn

B, T, D, H = 4, 1024, 1024, 16
DH = 64          # head dim
R = 256.0
HC = 8           # heads per core
E = 512          # channels per core (HC * DH)
N_CORES = 8
TCH = 512        # t-chunk width (PSUM f32 free-dim limit)
NT = T // TCH    # 2 t-chunks
ST = T // 128    # 8 s-tiles
DT = D // 128    # 8 d-tiles

SX = 32.0        # x pre-scale (|x| < 6 -> |x*SX| < 200, e4m3 max 448)
SW = 1024.0      # W pre-scale (|W| < 0.2)
DESCALE = 1.0 / (SX * SW)

_NC_CACHE = {}


def causal_width(st, tch):
    """Valid query-column width of block (s_tile=st, t_chunk=tch)."""
    delta = 128 * st - 512 * tch
    return max(0, min(TCH, delta + 128))


def span_width(st, tch, zlo):
    """Columns [0, m_w) where the span mask can differ from 1 (z >= zlo)."""
    delta = 128 * st - 512 * tch
    w = causal_width(st, tch)
    return max(0, min(w, delta + 127 - int(zlo)))


def dead_width(st, tch, zhi):
    """Columns [0, d_w) where the span mask is identically 0 (z <= zhi)."""
    delta = 128 * st - 512 * tch
    w = causal_width(st, tch)
    return max(0, min(w, int(delta - R - zhi)))


def _mt_layout(zlo, zhi, skip384):
    """(offsets dict {(k): (off, d_w, m_w)}, total cols) of the packed
    per-head span-ramp table; per-head stride is the total."""
    offs = {}
    off = 0
    for k in range(ST):
        m_w = span_width(k, 0, zlo)
        if m_w <= 0 or (k == 3 and skip384):
            continue
        d_w = dead_width(k, 0, zhi)
        if m_w <= d_w:
            continue
        offs[k] = (off, d_w, m_w)
        off += m_w - d_w
    return offs, off


def build_nc(zlo, zhi, skip384):
    key = (zlo, zhi, skip384)
    if key in _NC_CACHE:
        return _NC_CACHE[key]
    nc = bacc.Bacc("TRN2", target_bir_lowering=False, debug=False, num_devices=1)

    mt_offs, mt_cols = _mt_layout(zlo, zhi, skip384)

    # ---- DRAM parameters (per-core shards prepared on host) ----
    # hi/lo fp8 split tensors in [128, DT, free] mega-tile layout; DoubleRow
    # slots pair adjacent d-tiles of one product type (classic [P, 2, M]
    # stationary APs that walrus accepts; all slices DMA-contiguous)
    xh_d = nc.declare_dram_parameter("xh8", [128, DT, T], FP8, isOutput=False)
    xl_d = nc.declare_dram_parameter("xl8", [128, DT, T], FP8, isOutput=False)
    Wqh_d = nc.declare_dram_parameter("Wqh8", [128, DT, E], FP8, isOutput=False)
    Wql_d = nc.declare_dram_parameter("Wql8", [128, DT, E], FP8, isOutput=False)
    Wkh_d = nc.declare_dram_parameter("Wkh8", [128, DT, E], FP8, isOutput=False)
    Wkl_d = nc.declare_dram_parameter("Wkl8", [128, DT, E], FP8, isOutput=False)
    Wvh_d = nc.declare_dram_parameter("Wvh8", [128, DT, E], FP8, isOutput=False)
    Wvl_d = nc.declare_dram_parameter("Wvl8", [128, DT, E], FP8, isOutput=False)
    Wo_d = nc.declare_dram_parameter("Wo8", [128, D, 4], BF16, isOutput=False)
    # packed span-ramp tiles: mt8[p, h, off_k + j] = clamp((R+z_h-d)/R, 0, 1)
    # for d = 128k + p - (d_w + j); 0 where causal-invalid
    mt_d = nc.declare_dram_parameter("mt8", [128, HC, max(1, mt_cols)],
                                     F16, isOutput=False)
    # c01[s', k, j] = 1.0 if s' >= j else 0.0  (causal 0/1 for t' = 128k + j)
    c01_d = nc.declare_dram_parameter("c01", [128, 4, 128], F16, isOutput=False)
    yp_d = nc.declare_dram_parameter("yp", [T, D], F16, isOutput=True)

    with tile.TileContext(nc) as tc, ExitStack() as ctx:
        # ---------------- pools ----------------
        consts = ctx.enter_context(tc.tile_pool(name="consts", bufs=1))
        xp = ctx.enter_context(tc.tile_pool(name="xp", bufs=1))
        wp = ctx.enter_context(tc.tile_pool(name="wp", bufs=1))
        qkp = ctx.enter_context(tc.tile_pool(name="qkp", bufs=1))
        vp = ctx.enter_context(tc.tile_pool(name="vp", bufs=1))
        outp = ctx.enter_context(tc.tile_pool(name="outp", bufs=1))
        scr = ctx.enter_context(tc.tile_pool(name="scr", bufs=3))
        ysb = ctx.enter_context(tc.tile_pool(name="ysb", bufs=6))

        lead_ctx = ExitStack()
        ps_lead = lead_ctx.enter_context(
            tc.tile_pool(name="ps_lead", bufs=8, space="PSUM"))

        # ---------------- loads (few large DMAs) ----------------
        xh = xp.tile([128, DT, T], FP8, name="xh8")
        xl = xp.tile([128, DT, T], FP8, name="xl8")
        wqh = wp.tile([128, DT, E], FP8, name="wqh8")
        wql = wp.tile([128, DT, E], FP8, name="wql8")
        wkh = wp.tile([128, DT, E], FP8, name="wkh8")
        wkl = wp.tile([128, DT, E], FP8, name="wkl8")
        wvh = wp.tile([128, DT, E], FP8, name="wvh8")
        wvl = wp.tile([128, DT, E], FP8, name="wvl8")
        wo = wp.tile([128, D, 4], BF16, name="wo8")
        # DMA order tuned to the lead emission: V groups first (largest PE
        # block), then Q t-chunks, then K; mt/c01/wo only feed the
        # attention phase and stream last
        # first two tensors stream in dt-halves so the first V matmuls
        # start ~1.5us sooner
        nc.sync.dma_start(xh[:, 0:4, 0:TCH], xh_d[:, 0:4, 0:TCH])
        nc.sync.dma_start(wvh[:, 0:4, :], Wvh_d[:, 0:4, :])
        nc.sync.dma_start(xh[:, 4:DT, 0:TCH], xh_d[:, 4:DT, 0:TCH])
        nc.sync.dma_start(wvh[:, 4:DT, :], Wvh_d[:, 4:DT, :])
        nc.sync.dma_start(xl[:, :, 0:TCH], xl_d[:, :, 0:TCH])
        nc.sync.dma_start(wvl[:], Wvl_d[:, :, :])
        nc.sync.dma_start(wqh[:], Wqh_d[:, :, :])
        nc.sync.dma_start(wql[:], Wql_d[:, :, :])
        nc.sync.dma_start(xh[:, :, TCH:T], xh_d[:, :, TCH:T])
        nc.sync.dma_start(wkh[:], Wkh_d[:, :, :])
        nc.sync.dma_start(xl[:, :, TCH:T], xl_d[:, :, TCH:T])
        nc.sync.dma_start(wkl[:], Wkl_d[:, :, :])
        c01_sb = consts.tile([128, 4, 128], F16, tag="c01")
        nc.sync.dma_start(c01_sb[:], c01_d[:, :, :])
        # mt sliced per head-pair: with attention starting ~21.5us, pair
        # (0,0)'s first ramp block would otherwise race the full-table DMA
        mt_sb = consts.tile([128, HC, max(1, mt_cols)], F16, tag="mt8")
        nc.sync.dma_start(mt_sb[:, 0:2, :], mt_d[:, 0:2, :])
        nc.sync.dma_start(mt_sb[:, 2:4, :], mt_d[:, 2:4, :])
        nc.sync.dma_start(wo[:], Wo_d[:, :, :])
        nc.sync.dma_start(mt_sb[:, 4:6, :], mt_d[:, 4:6, :])
        nc.sync.dma_start(mt_sb[:, 6:8, :], mt_d[:, 6:8, :])

        # ---------------- Q/K projections (transposed layout) ----------------
        # QT[e, t] = sum_d W[d, e] * xT[d, t] in fp8 DoubleRow 3-term:
        # per dtile (wh)x(xh,xl), per dtile-pair (wl_d,wl_d+1)x(xh_d,xh_d+1)
        qt_sb = [qkp.tile([128, T], BF16, tag="qt", name=f"qt{i}", bufs=4)
                 for i in range(4)]
        kt_sb = [qkp.tile([128, T], BF16, tag="kt", name=f"kt{i}", bufs=4)
                 for i in range(4)]

        def proj_mms(ps, w_hl, et, t0, t1):
            w_hi, w_lo = w_hl
            eb = 128 * et
            w_cols = t1 - t0
            for i, (wt, xt) in enumerate(
                    ((w_hi, xh), (w_hi, xl), (w_lo, xh))):
                for dt_i in range(0, DT, 2):
                    nc.tensor.matmul(
                        ps[:, 0:w_cols],
                        wt[:, dt_i:dt_i + 2, eb:eb + 128],
                        xt[:, dt_i:dt_i + 2, t0:t1],
                        start=(i == 0 and dt_i == 0),
                        stop=(i == 2 and dt_i == DT - 2), perf_mode=DR)

        def emit_proj(dst, w8, et, t0, t1, copy_eng="act", pool=None):
            pool = pool or ps_lead
            ps = pool.tile([128, TCH], F32, tag="pj", name=f"pj{et}_{t0}",
                           padded_shape=[128, TCH])
            proj_mms(ps, w8, et, t0, t1)
            if copy_eng == "act":
                nc.scalar.copy(dst[et][:, t0:t1], ps[:, 0:t1 - t0])
            else:
                nc.vector.tensor_copy(dst[et][:, t0:t1], ps[:, 0:t1 - t0])

        WQ = (wqh, wql)
        WK = (wkh, wkl)

        # ---------------- V (natural layout, ones-augmented) ----------------
        # v_aug[st][p, h, 0:64] = V_raw[128*st+p, 64h+j] (scaled SX*SW);
        # v_aug[st][p, h, 64:128] = 1 (exact denominator rows; the V descale
        # folds into the finalize multiplier)
        v_aug = [None] * ST

        def emit_v(st, pool=None, copy_eng="dve"):
            pool = pool or ps_lead
            va = vp.tile([128, HC, 2 * DH], BF16, tag="vaug", bufs=ST,
                         name=f"vaug{st}")
            nc.gpsimd.memset(va[:, :, DH:2 * DH], 1.0)
            ps = pool.tile([128, E], F32, tag="pj", name=f"pjv{st}")
            sb = 128 * st
            for i, (xt, wt) in enumerate(
                    ((xh, wvh), (xl, wvh), (xh, wvl))):
                for dt_i in range(0, DT, 2):
                    nc.tensor.matmul(
                        ps[:],
                        xt[:, dt_i:dt_i + 2, sb:sb + 128],
                        wt[:, dt_i:dt_i + 2, :],
                        start=(i == 0 and dt_i == 0),
                        stop=(i == 2 and dt_i == DT - 2), perf_mode=DR)
            ceng = nc.scalar.copy if copy_eng == "act" else nc.vector.tensor_copy
            ceng(va[:, :, 0:DH], ps[:].rearrange("p (h d) -> p h d", h=HC))
            v_aug[st] = va

        # lead emission ordered by DMA arrival: V st0-5 first (x + wv),
        # then Q chunks, then K et0 only -- K et1, V6, V7 move into pair
        # (0,0)'s fillers so attention starts ~5us earlier and that work
        # lands in the attention phase's stall pockets
        for st in range(4):
            emit_v(st, copy_eng=("dve" if st < 2 else "act"))
        emit_proj(qt_sb, WQ, 0, 0, TCH)
        emit_proj(qt_sb, WQ, 1, 0, TCH)
        emit_proj(qt_sb, WQ, 0, TCH, T)
        emit_proj(qt_sb, WQ, 1, TCH, T)
        # K et0 copies go on DVE: they land right at attention start and
        # must not queue ahead of the first exp ops on ACT
        for tch in range(NT):
            emit_proj(kt_sb, WK, 0, TCH * tch, TCH * (tch + 1),
                      copy_eng="dve")

        # spare projection work at the lead tail keeps PE busy while the
        # first score block's psum bank clears its lead-phase WAR; two
        # half-width groups make the last copy (the WAR reader) short
        ps_sp1 = ps_lead.tile([128, 256], F32, tag="pj", name="pjsp1",
                              padded_shape=[128, TCH])
        proj_mms(ps_sp1, WQ, 2, 0, 256)
        nc.scalar.copy(qt_sb[2][:, 0:256], ps_sp1[:, 0:256])
        ps_sp2 = ps_lead.tile([128, 256], F32, tag="pj", name="pjsp2",
                              padded_shape=[128, TCH])
        proj_mms(ps_sp2, WQ, 2, 256, TCH)
        nc.vector.tensor_copy(qt_sb[2][:, 256:TCH], ps_sp2[:, 0:256])

        lead_ctx.close()
        attn_ctx = ExitStack()
        ps_sc = attn_ctx.enter_context(
            tc.tile_pool(name="ps_sc", bufs=2, space="PSUM"))
        ps_out = attn_ctx.enter_context(
            tc.tile_pool(name="ps_out", bufs=2, space="PSUM"))
        ps_fill = attn_ctx.enter_context(
            tc.tile_pool(name="ps_fill", bufs=2, space="PSUM"))

        def proj_fillers(specs):
            """Filler closures whose psum->SBUF copy is deferred one slot so
            it queues behind the current block's exp/mask, not ahead."""
            fs = []
            pend = [None]

            def make(dst, w8, et, tch):
                def f():
                    ps = ps_fill.tile([128, TCH], F32, tag="pj",
                                      name=f"pjf{et}_{tch}",
                                      padded_shape=[128, TCH])
                    proj_mms(ps, w8, et, TCH * tch, TCH * (tch + 1))
                    prev, pend[0] = pend[0], (
                        lambda: nc.vector.tensor_copy(
                            dst[et][:, TCH * tch:TCH * (tch + 1)], ps[:]))
                    if prev is not None:
                        prev()
                return f

            for dst, w8, et, tch in specs:
                fs.append(make(dst, w8, et, tch))

            def flush():
                if pend[0] is not None:
                    pend[0]()
                    pend[0] = None
            fs.append(flush)
            return fs

        # ---------------- attention ----------------
        # out_pair[j][tch] holds heads 2j (parts 0:64) and 2j+1 (parts 64:128)
        out_pair = [[outp.tile([128, TCH], BF16, tag="out", bufs=8,
                               name=f"op{j}_{c}") for c in range(NT)]
                    for j in range(4)]

        # exp absorbs the Q/K descales: p = exp(s_raw / (8 * (SX*SW)^2))
        EXP_SCALE = 1.0 / (8.0 * (SX * SW) ** 2)

        def attn_pair(tch, j, v_prefetch=False, fillers=(), prev_fin=None,
                      split_fin=False, split_exp=False, fin_idx=0):
            """Attention for head pair (2j, 2j+1); both share et=j.

            Scores for the two heads go into one 2-bank psum pair-tile so a
            single exp covers both. Scores run one block ahead of attn@V so
            the exp+mask chain hides under PE work. The previous pair's
            out-division (prev_fin) is emitted after this pair's first score
            block so it does not wedge ahead of this pair's mask ops in the
            DVE queue. Returns this pair's finalize closure.
            """
            first_st = 4 * tch
            heads = (2 * j, 2 * j + 1)
            pouts = [ps_out.tile([128, TCH], F32, tag="pout",
                                 name=f"pout{h}_{tch}") for h in heads]
            fillers = list(fillers)
            p_tiles = {}

            def block_ranges(st):
                w = causal_width(st, tch)
                d_w = dead_width(st, tch, zhi)
                return [(d_w, w)]

            def emit_sc(st):
                w = causal_width(st, tch)
                d_w = dead_width(st, tch, zhi)
                k = st - first_st  # delta = 128*k
                sc_hp = ps_sc.tile([128, 2, TCH], F32, tag="sc",
                                   name=f"sc{j}_{st}")
                p_hp = scr.tile([128, 2, TCH], BF16, tag="p", bufs=12,
                                name=f"p{j}_{st}")
                for c0, c1 in block_ranges(st):
                    for i, h in enumerate(heads):
                        hp = (h % 2) * 64
                        nc.tensor.matmul(
                            sc_hp[:, i, c0:c1],
                            kt_sb[j][hp:hp + DH, 128 * st:128 * (st + 1)],
                            qt_sb[j][hp:hp + DH,
                                     TCH * tch + c0:TCH * tch + c1],
                            start=True, stop=True)
                    nc.scalar.activation(
                        p_hp[:, :, c0:c1], sc_hp[:, :, c0:c1],
                        mybir.ActivationFunctionType.Exp, scale=EXP_SCALE)
                    # one mask op covers BOTH heads ([128, 2, w] tiles and
                    # mt rows are head-pair adjacent) -> half the op count
                    # and half the chain latency
                    if k <= 3:
                        # diagonal block: causal zeroing on [128k, w)
                        d0 = 128 * k
                        v0, v1 = max(c0, d0), min(c1, w)
                        if v1 > v0:
                            nc.gpsimd.tensor_mul(
                                p_hp[:, :, v0:v1], p_hp[:, :, v0:v1],
                                c01_sb[:, k:k + 1, v0 - d0:v1 - d0]
                                .broadcast_to([128, 2, v1 - v0]))
                    if k in mt_offs:
                        # span mask: p *= mt (host-precomputed clamp);
                        # ramp bands are wide -> DVE (f16 2x), not gpsimd
                        off, mt_d, m_w = mt_offs[k]
                        v0, v1 = max(c0, mt_d), min(c1, m_w)
                        if v1 > v0:
                            nc.vector.tensor_mul(
                                p_hp[:, :, v0:v1], p_hp[:, :, v0:v1],
                                mt_sb[:, 2 * j:2 * j + 2,
                                      off + v0 - mt_d:off + v1 - mt_d])
                p_tiles[st] = p_hp

            def emit_av(st):
                av_first = ST - 1 if tch == 1 else first_st
                av_last = first_st if tch == 1 else ST - 1
                for c0, c1 in block_ranges(st):
                    for i, h in enumerate(heads):
                        nc.tensor.matmul(
                            pouts[i][:, c0:c1], v_aug[st][:, h, :],
                            p_tiles[st][:, i, c0:c1],
                            start=(st == av_first), stop=(st == av_last),
                            skip_group_check=True)

            # tch=1 pairs run largest-first: the pair ends on its smallest
            # exp, so the next pair's score banks are freed ~3x sooner at
            # the handoff (tch=0 keeps ascending; v-prefetch requires it and
            # its wide late blocks have no filler cover early)
            desc = tch == 1
            order = (list(range(ST - 1, first_st - 1, -1)) if desc
                     else list(range(first_st, ST)))
            fin_early, fin_late = None, None
            if prev_fin is not None:
                if fin_idx == 'split':
                    fin_early, fin_late = prev_fin.split()
                elif fin_idx == 0:
                    fin_early = prev_fin
                else:
                    fin_late = prev_fin
            emit_sc(order[0])
            if fin_early is not None:
                fin_early()
            for idx in range(1, len(order)):
                st = order[idx]
                if v_prefetch and st + 3 < ST and v_aug[st + 3] is None:
                    emit_v(st + 3, pool=ps_fill)
                if fillers:
                    fillers.pop(0)()
                emit_sc(st)
                # deferred finalize part: must precede this pair's first
                # attn@V (idx==2), which reuses the previous pout banks
                if idx == 2 and fin_late is not None:
                    fin_late()
                if idx >= 2:
                    emit_av(order[idx - 2])
            if fillers:
                fillers.pop(0)()
            emit_av(order[-2])
            emit_av(order[-1])

            def div_chunk(rws, c0, c1):
                for i, h in enumerate(heads):
                    hp = (h % 2) * 64
                    nc.vector.scalar_tensor_tensor(
                        out_pair[j][tch][hp:hp + DH, c0:c1],
                        pouts[i][0:DH, c0:c1], DESCALE, rws[i][:, c0:c1],
                        op0=mybir.AluOpType.mult,
                        op1=mybir.AluOpType.mult)

            def recips():
                # rows 0:64 numerator (scaled SX*SW); rows 64:128 denominator
                # W (unscaled); the division multiplier folds the V descale
                rws = []
                for i, h in enumerate(heads):
                    rw = scr.tile([DH, TCH], F32, tag="rw", bufs=8,
                                  name=f"rw{h}")
                    with nc.allow_low_precision(reason="denom recip bf16"):
                        nc.vector.reciprocal(rw[:], pouts[i][DH:2 * DH, :])
                    rws.append(rw)
                return rws

            def finalize():
                rws = recips()
                chunks = (0, 256, TCH) if split_fin else (0, TCH)
                for c0, c1 in zip(chunks[:-1], chunks[1:]):
                    div_chunk(rws, c0, c1)

            def finalize_split():
                # (early, late): early covers cols 0:256 (all the next
                # pair's y-filler reads for tt 0/1); late defers the rest
                st_ = {}

                def early():
                    st_['rws'] = recips()
                    div_chunk(st_['rws'], 0, 256)

                def late():
                    div_chunk(st_['rws'], 256, TCH)
                return early, late

            finalize.split = finalize_split
            return finalize

        copy_rot = [0]

        def y_group(tt, nch, pool, engines=("act", "pool"), yo=None):
            """One output-projection psum group; DMA fires unless yo is a
            shared per-tt tile whose DMA the caller batches."""
            tch = tt // 4
            toff = 128 * tt - TCH * tch
            yps = pool.tile([128, TCH], F32, tag="pj", name=f"y{tt}_{nch}")
            for j in range(4):
                nc.tensor.matmul(
                    yps[:],
                    out_pair[j][tch][:, toff:toff + 128],
                    wo[:, TCH * nch:TCH * (nch + 1), j],
                    start=(j == 0), stop=(j == 3))
            own_dma = yo is None
            if own_dma:
                yo_sl = ysb.tile([128, TCH], F16, tag="y", bufs=8)
            else:
                yo_sl = yo[:, TCH * nch:TCH * (nch + 1)]
            eng = engines[copy_rot[0] % len(engines)]
            copy_rot[0] += 1
            if eng == "dve":
                nc.vector.tensor_copy(yo_sl[:] if own_dma else yo_sl, yps[:])
            else:
                nc.scalar.copy(yo_sl[:] if own_dma else yo_sl, yps[:])
            if own_dma:
                nc.sync.dma_start(
                    yp_d[128 * tt:128 * (tt + 1), TCH * nch:TCH * (nch + 1)],
                    yo_sl[:])

        def y_fillers(tts):
            fs = []
            pend = [None]

            def make(tt, nch):
                def f():
                    tch0 = tt // 4
                    toff = 128 * tt - TCH * tch0
                    yps = ps_fill.tile([128, TCH], F32, tag="pj",
                                       name=f"y{tt}_{nch}")
                    for j in range(4):
                        nc.tensor.matmul(
                            yps[:],
                            out_pair[j][tch0][:, toff:toff + 128],
                            wo[:, TCH * nch:TCH * (nch + 1), j],
                            start=(j == 0), stop=(j == 3))

                    def copy_dma():
                        yo = ysb.tile([128, TCH], F16, tag="y", bufs=8)
                        nc.vector.tensor_copy(yo[:], yps[:])
                        nc.sync.dma_start(
                            yp_d[128 * tt:128 * (tt + 1),
                                 TCH * nch:TCH * (nch + 1)], yo[:])
                    prev, pend[0] = pend[0], copy_dma
                    if prev is not None:
                        prev()
                return f

            for tt in tts:
                for nch in range(NT):
                    fs.append(make(tt, nch))

            def flush():
                if pend[0] is not None:
                    pend[0]()
                    pend[0] = None
            fs.append(flush)
            return fs

        kf = proj_fillers([(kt_sb, WK, 1, 0), (kt_sb, WK, 1, 1)])
        f0 = [lambda: emit_v(4, pool=ps_fill), kf[0],
              lambda: emit_v(5, pool=ps_fill), kf[1],
              lambda: emit_v(6, pool=ps_fill), kf[2],
              lambda: emit_v(7, pool=ps_fill)]
        f1 = proj_fillers([(qt_sb, WQ, 2, 1), (kt_sb, WK, 2, 0),
                           (kt_sb, WK, 2, 1)])
        f2 = proj_fillers([(qt_sb, WQ, 3, 0), (qt_sb, WQ, 3, 1),
                           (kt_sb, WK, 3, 0)])
        f3 = proj_fillers([(kt_sb, WK, 3, 1)])
        fin = attn_pair(0, 0, fillers=f0)
        fin = attn_pair(0, 1, fillers=f1, prev_fin=fin, fin_idx=2)
        fin = attn_pair(0, 2, fillers=f2, prev_fin=fin, fin_idx=2)
        fin = attn_pair(0, 3, fillers=f3, prev_fin=fin, fin_idx=2)
        fin = attn_pair(1, 0, fillers=y_fillers([0]), prev_fin=fin,
                        fin_idx='split')
        fin = attn_pair(1, 1, fillers=y_fillers([1]), prev_fin=fin, fin_idx=2)
        fin = attn_pair(1, 2, fillers=y_fillers([2]), prev_fin=fin, fin_idx=2)
        fin = attn_pair(1, 3, fillers=y_fillers([3]), prev_fin=fin, fin_idx=2,
                        split_fin=True)
        fin()
        attn_ctx.close()
        with tc.tile_pool(name="ps_tail", bufs=4, space="PSUM") as ps_tail:
            # per-tt batched DMAs: 4 issues total, each overlapping the
            # remaining matmuls; the last chain is copy + one issue + xfer
            for tt in range(4, 7):
                yo = ysb.tile([128, D], F16, tag="yb", bufs=3)
                for nch in range(NT):
                    y_group(tt, nch, ps_tail,
                            engines=("act", "dve"), yo=yo)
                nc.sync.dma_start(yp_d[128 * tt:128 * (tt + 1), :], yo[:])
            for nch in range(NT):
                y_group(7, nch, ps_tail, engines=("act", "dve"))

    nc.compile()
    _NC_CACHE[key] = nc
    return nc


def _split_pack(a, scale):
    """[D, F] f32 -> (hi, lo) [128, DT, F] fp8e4 mega-tiles."""
    Dd, F = a.shape
    s = (a * scale).astype(np.float32)
    hi = s.astype(E4NP)
    lo = (s - hi.astype(np.float32)).astype(E4NP)
    def pack(m):
        return np.ascontiguousarray(
            m.reshape(Dd // 128, 128, F).transpose(1, 0, 2))
    return pack(hi), pack(lo)


def _pack_dtiles(w):
    """[D, F] -> [128, F, DT] mega-tile (partition, inner, d-tile)."""
    Dd, F = w.shape
    return np.ascontiguousarray(
        w.reshape(Dd // 128, 128, F).transpose(1, 2, 0))


def _prep_core_inputs(x, Wq, Wk, Wv, Wo, z, zlo, zhi, skip384):
    bf = ml_dtypes.bfloat16
    c01 = _make_c01()
    w_splits = []
    for W in (Wq, Wk, Wv):
        w_splits.append([
            _split_pack(np.ascontiguousarray(W[E * g:E * (g + 1), :].T), SW)
            for g in range(2)])
    wo_packs = [
        _pack_dtiles(np.ascontiguousarray(Wo[:, E * g:E * (g + 1)].T)).astype(bf)
        for g in range(2)]
    x_splits = [_split_pack(np.ascontiguousarray(x[b].T), SX)
                for b in range(B)]
    in_maps = []
    for c in range(N_CORES):
        b, g = c // 2, c % 2
        in_maps.append({
            "c01": c01,
            "xh8": x_splits[b][0],
            "xl8": x_splits[b][1],
            "Wqh8": w_splits[0][g][0], "Wql8": w_splits[0][g][1],
            "Wkh8": w_splits[1][g][0], "Wkl8": w_splits[1][g][1],
            "Wvh8": w_splits[2][g][0], "Wvl8": w_splits[2][g][1],
            "Wo8": wo_packs[g],
            "mt8": _make_mt(z[b, HC * g:HC * (g + 1)], zlo, zhi, skip384),
        })
    return in_maps


def _make_c01():
    sp = np.arange(128, dtype=np.float32)[:, None]
    jp = np.arange(128, dtype=np.float32)[None, :]
    m = (sp - jp >= 0).astype(np.float16)
    return np.ascontiguousarray(
        np.broadcast_to(m[:, None, :], (128, 4, 128))).astype(np.float16)


def _make_mt(z_h, zlo, zhi, skip384):
    """Per-head packed span-ramp tiles [128, HC, mt_cols] f16."""
    mt_offs, mt_cols = _mt_layout(zlo, zhi, skip384)
    out = np.zeros((128, HC, max(1, mt_cols)), np.float16)
    sp = np.arange(128, dtype=np.float32)[:, None]
    for k, (off, d_w, m_w) in mt_offs.items():
        tp = np.arange(d_w, m_w, dtype=np.float32)[None, :]
        d = 128.0 * k + sp - tp
        for h in range(HC):
            ramp = np.clip((R + z_h[h] - d) / R, 0.0, 1.0)
            ramp = np.where(d < 0, 0.0, ramp)  # causal-invalid -> 0
            out[:, h, off:off + m_w - d_w] = ramp.astype(np.float16)
    return out


def _nc_params(x, Wspan, bspan):
    """Span bounds from host-exact z; specializes mask widths per call."""
    x = np.asarray(x, np.float32)
    Wspan = np.asarray(Wspan, np.float32)
    bspan = np.asarray(bspan, np.float32)
    logits = x.mean(axis=1) @ Wspan.T + bspan
    z = T / (1.0 + np.exp(-logits))
    # exact bounds: the device uses host-built mask tables, so the widths
    # only need zlo <= z.min (ramp-band start) and zhi > z.max (dead cols)
    zlo = max(0, int(z.min()))
    zhi = int(z.max()) + 1
    # skipping the delta=384 ramp (dist in (z, 511]) perturbs <= (511-z)/R
    # of the weight on a sliver of columns; safe when z_min >= 491
    skip384 = bool(z.min() >= 491.0)
    return z, zlo, zhi, skip384


def kernel(x, Wq, Wk, Wv, Wo, bo, Wspan, bspan):
    x = np.asarray(x, np.float32)
    Wq = np.asarray(Wq, np.float32)
    Wk = np.asarray(Wk, np.float32)
    Wv = np.asarray(Wv, np.float32)
    Wo = np.asarray(Wo, np.float32)
    bo = np.asarray(bo, np.float32)
    Wspan = np.asarray(Wspan, np.float32)
    bspan = np.asarray(bspan, np.float32)

    z, zlo, zhi, skip384 = _nc_params(x, Wspan, bspan)
    nc = build_nc(zlo, zhi, skip384)
    in_maps = _prep_core_inputs(x, Wq, Wk, Wv, Wo, z, zlo, zhi, skip384)
    res = run_bass_kernel_spmd(nc, in_maps, core_ids=list(range(N_CORES)))
    y = np.empty((B, T, D), np.float32)
    for b in range(B):
        y[b] = (res.results[2 * b]["yp"].astype(np.float32)
                + res.results[2 * b + 1]["yp"].astype(np.float32) + bo)
    return y
